# revision 3
# baseline (speedup 1.0000x reference)
"""Trainium2 Bass kernel for BinaryDecorator:
    out = (sign(x) @ sign(W).T + b) * mean(|x|)

x: [524288, 128] fp32, W: [128, 128] fp32, b: [128] fp32.

Strategy (8 NeuronCores, data-parallel over rows of x):
  Phase 1 (per core, 65536 rows): stream x from HBM once. For each
    128x128 tile: PE-transpose it to PSUM, take Sign on ScalarE while
    copying PSUM -> SBUF as fp8 (+-1 exact) into a persistent 8MB SBUF
    stash laid out [k=128 partitions, n free] (matmul-ready). VectorE
    accumulates per-partition sums of |x| on the natural-layout tile.
  Phase 2: local reduce of |x| partials, AllReduce([128,1]) across the
    8 cores, cross-partition reduce+broadcast on GpSimd, scale by
    1/(N*D) -> mean in every partition. Bias row is scaled by mean and
    partition-broadcast to a [128,128] tile.
  Phase 3: for each row tile, matmul(lhsT=stash slice fp8, rhs=sign(W).T
    fp8) -> PSUM [n,128] (exact integer counts), ScalarE copy*mean,
    VectorE add bias*mean, DMA out.

HBM traffic per core = 32MB read + 32MB write = the roofline minimum.
"""

import sys

for _p in ("/opt/trn_rl_repo",):
    if _p not in sys.path:
        sys.path.append(_p)

import numpy as np

import concourse.bass as bass
import concourse.mybir as mybir
import concourse.tile as tile
from concourse import bacc, bass_isa, bass_utils
from concourse.bass import ds
from concourse.masks import make_identity

N_TOTAL = 524288
D = 128
NCORES = 8
N_PER_CORE = N_TOTAL // NCORES
P = 128
T_SUB = 8  # 128-row subtiles per iteration (1024 rows / 512KB per DMA)

F32 = mybir.dt.float32
FP8 = mybir.dt.float8e4
AF = mybir.ActivationFunctionType


def emit(tc, out_ap, x_ap, w_ap, b_ap, total_elems, ncores):
    nc = tc.nc
    n_rows = x_ap.shape[0]
    rows_per_iter = T_SUB * P
    assert n_rows % rows_per_iter == 0
    iters = n_rows // rows_per_iter

    x_view = x_ap.rearrange("(i t p) k -> i p t k", t=T_SUB, p=P)
    out_view = out_ap.rearrange("(i t p) k -> i p t k", t=T_SUB, p=P)

    import contextlib

    with contextlib.ExitStack() as ctx:
        const = ctx.enter_context(tc.tile_pool(name="const", bufs=1))
        stash = ctx.enter_context(tc.tile_pool(name="stash", bufs=1))
        xin = ctx.enter_context(tc.tile_pool(name="xin", bufs=3))
        outp = ctx.enter_context(tc.tile_pool(name="outp", bufs=3))
        ptp = ctx.enter_context(tc.tile_pool(name="ptp", bufs=4, space="PSUM"))
        pmm = ctx.enter_context(tc.tile_pool(name="pmm", bufs=4, space="PSUM"))
        dram = ctx.enter_context(tc.tile_pool(name="dram", bufs=1, space="DRAM"))

        identity = const.tile([P, P], F32, name="identity")
        make_identity(nc, identity)

        # --- weights: sign(W)^T as fp8, laid out [k, o] ---
        w_nat = const.tile([P, P], F32, name="w_nat")
        nc.sync.dma_start(w_nat[:], w_ap)
        psum_w = ptp.tile([P, P], F32, name="tp", tag="tp")
        nc.tensor.transpose(psum_w[:], w_nat[:], identity[:])
        wsT = const.tile([P, P], FP8, name="wsT")
        nc.scalar.activation(wsT[:], psum_w[:], AF.Sign)

        bias_row = const.tile([1, D], F32, name="bias_row")
        nc.sync.dma_start(bias_row[:], b_ap[None, :])

        xbT = stash.tile([P, n_rows], FP8, name="xbT")
        acc_all = const.tile([P, iters], F32, name="acc_all")

        # --- phase 1: stream x, stash sign(x)^T, accumulate |x| ---
        for i in range(iters):
            x_nat = xin.tile([P, T_SUB, P], F32, name="x_nat", tag="x_nat")
            nc.sync.dma_start(x_nat[:], x_view[i])
            for t in range(T_SUB):
                tp = ptp.tile([P, P], F32, name="tp", tag="tp")
                nc.tensor.transpose(tp[:], x_nat[:, t, :], identity[:])
                col = (i * T_SUB + t) * P
                nc.scalar.activation(xbT[:, ds(col, P)], tp[:], AF.Sign)
            nc.vector.tensor_reduce(
                acc_all[:, i : i + 1],
                x_nat[:],
                axis=mybir.AxisListType.XY,
                op=mybir.AluOpType.add,
                apply_absolute_value=True,
            )

        # --- phase 2: global mean(|x|) ---
        acc_col = const.tile([P, 1], F32, name="acc_col")
        nc.vector.tensor_reduce(
            acc_col[:],
            acc_all[:],
            axis=mybir.AxisListType.X,
            op=mybir.AluOpType.add,
        )
        cc_in = dram.tile([P, 1], F32, name="cc_in")
        cc_out = dram.tile([P, 1], F32, name="cc_out", addr_space="Shared")
        nc.sync.dma_start(cc_in[:], acc_col[:])
        nc.gpsimd.collective_compute(
            "AllReduce",
            mybir.AluOpType.add,
            replica_groups=[list(range(ncores))],
            ins=[cc_in[:].opt()],
            outs=[cc_out[:].opt()],
        )
        allred = const.tile([P, 1], F32, name="allred")
        nc.sync.dma_start(allred[:], cc_out[:])
        tot = const.tile([P, 1], F32, name="tot")
        nc.gpsimd.partition_all_reduce(
            tot[:], allred[:], channels=P, reduce_op=bass_isa.ReduceOp.add
        )
        mean_col = const.tile([P, 1], F32, name="mean_col")
        nc.scalar.mul(mean_col[:], tot[:], 1.0 / float(total_elems))

        # bias * mean, broadcast to all partitions
        bias_s = const.tile([1, D], F32, name="bias_s")
        nc.vector.tensor_scalar_mul(bias_s[:], bias_row[:], mean_col[0:1, :])
        bias_bb = const.tile([P, D], F32, name="bias_bb")
        nc.gpsimd.partition_broadcast(bias_bb[:], bias_s[:])

        # --- phase 3: matmul, scale, bias, write out ---
        for i in range(iters):
            out_sb = outp.tile([P, T_SUB, D], F32, name="out_sb", tag="out_sb")
            for t in range(T_SUB):
                mm = pmm.tile([P, D], F32, name="mm", tag="mm")
                col = (i * T_SUB + t) * P
                nc.tensor.matmul(
                    mm[:], xbT[:, ds(col, P)], wsT[:], start=True, stop=True
                )
                nc.scalar.activation(
                    out_sb[:, t, :], mm[:], AF.Copy, scale=mean_col[:]
                )
                nc.vector.tensor_tensor(
                    out_sb[:, t, :], out_sb[:, t, :], bias_bb[:], mybir.AluOpType.add
                )
            nc.sync.dma_start(out_view[i], out_sb[:])


def build_module(n_per_core=N_PER_CORE, ncores=NCORES, repeats=1):
    nc = bacc.Bacc(
        "TRN2",
        target_bir_lowering=False,
        debug=False,
        enable_asserts=False,
        num_devices=ncores,
    )
    x_t = nc.dram_tensor("x", [n_per_core, D], F32, kind="ExternalInput")
    w_t = nc.dram_tensor("weight", [D, D], F32, kind="ExternalInput")
    b_t = nc.dram_tensor("bias", [D], F32, kind="ExternalInput")
    o_t = nc.dram_tensor("out", [n_per_core, D], F32, kind="ExternalOutput")
    with tile.TileContext(nc) as tc:
        for r in range(repeats):
            if r:
                tc.strict_bb_all_engine_barrier()
            emit(
                tc,
                o_t.ap(),
                x_t.ap(),
                w_t.ap(),
                b_t.ap(),
                total_elems=n_per_core * ncores * D,
                ncores=ncores,
            )
    nc.compile()
    return nc


_CACHE = {}


def get_module(n_per_core=N_PER_CORE, ncores=NCORES, repeats=1):
    key = (n_per_core, ncores, repeats)
    if key not in _CACHE:
        _CACHE[key] = build_module(n_per_core, ncores, repeats)
    return _CACHE[key]


def kernel(x, weight, bias):
    x = np.ascontiguousarray(np.asarray(x, dtype=np.float32))
    weight = np.ascontiguousarray(np.asarray(weight, dtype=np.float32))
    bias = np.ascontiguousarray(np.asarray(bias, dtype=np.float32))
    assert x.shape == (N_TOTAL, D), x.shape

    nc = get_module()
    in_maps = [
        {
            "x": x[c * N_PER_CORE : (c + 1) * N_PER_CORE],
            "weight": weight,
            "bias": bias,
        }
        for c in range(NCORES)
    ]
    res = bass_utils.run_bass_kernel_spmd(nc, in_maps, core_ids=list(range(NCORES)))
    return np.concatenate([r["out"] for r in res.results], axis=0)


if __name__ == "__main__":
    import time

    t0 = time.time()
    nc = build_module()
    print("build+compile OK in", time.time() - t0, "s")
